# revision 30
# baseline (speedup 1.0000x reference)
"""Trainium2 Bass kernel for a 2-layer ConvLSTM block (B=4,T=8,64x64,C=F=32).

Sharding: 8 cores = batch(4) x H-halves(2). Each core computes 40 of 64 rows
(32 output rows + 8 redundant "ghost" rows) so NO cross-core communication is
needed. Bottom-half cores receive vertically flipped inputs and kh-flipped
conv weights so all 8 cores run an identical SPMD program.

Device algorithm per (layer, step) — "pixel-major" scheme:
  - 3x3 convs as DATA-STATIONARY matmuls: lhsT = im2col 2-row block
    [K<=99, M=128 px], rhs = weights [K, N=128 gates]. z comes out of the PE
    already pixel-major: psum [128 px, 128 gc] per block, so gate math needs
    NO partition reshuffle (no fold DMAs at all).
  - z staged to SBUF as zs [128 q, 20 b, 128 gc] (5 copies of [128,512]
    spread over ACT/DVE/Pool engines).
  - gates: hard_sigmoid = ACT Relu(0.2z+0.5) + DVE min(.,1); tanh on ACT;
    elementwise muls/adds on DVE; everything fp16, c-state fp16.
  - h [128 q, 640(b,c)] is transposed by ONE XBAR dma_start_transpose to
    hT [32 c, 20 b, 128 q] (channel-major, pixels contiguous), then 3 DMAs
    write the dx-shifted im2col blocks of the recurrent conv buffer.
  - BN2 is folded into the layer-2 input-conv weights: rows 0:96 of w2 are
    scaled by s2 per input channel; 3 constant "mask" rows (96:99) in the
    ys1h buffer inject the t2 shift with correct SAME-padding masking, and
    bias2 rides the center-tap mask row. Layer 2 reads layer 1's h buffer
    directly — no BN activation op and no separate ys2x buffer.
  - residual h2+h1 kept entirely in SBUF.
Host does: BN1 fold into x, im2col staging of x, weight reshuffling
(gate-reorder + bias row + vertical flip + BN2 fold), output decode.
"""
import numpy as np

import concourse.bass as bass
import concourse.tile as tile
from concourse import mybir
from concourse.bass_utils import run_bass_kernel_spmd

F32 = mybir.dt.float32
F16 = mybir.dt.float16
AF = mybir.ActivationFunctionType

B, T, H, W, C, F = 4, 8, 64, 64, 32, 32
L = 2
BN_EPS = 1e-3
R = 40          # compute rows per core
RD = 41         # data rows of x each core needs (R + 1 halo)
RR = 42         # padded rows in im2col buffers
WW = 64         # row width (no col pads; dx shifts are in the blocks)
NB = 20         # 2-row pixel blocks per step (M=128 px each)
NCORES = 8

_PROG = {}


def _split_excess_waits(nc, max_waits=1):
    """This walrus rejects >1 sync-wait per instruction on some engines; move
    excess waits onto NoOps inserted just before, on the same engine."""
    for fn in nc.m.functions:
        for bb in fn.blocks:
            new_insts = []
            for inst in bb.instructions:
                si = inst.sync_info
                waits = list(si.on_wait) if si and si.on_wait else []
                if len(waits) > max_waits:
                    k = 0
                    while len(waits) - k > max_waits:
                        chunk = waits[k:k + max_waits]
                        k += max_waits
                        new_insts.append(mybir.InstNoOp(
                            name=f"waitsplit_{inst.name}_{k}",
                            engine=inst.engine,
                            sync_info=mybir.SyncInfo(on_wait=list(chunk),
                                                     on_update=[]),
                        ))
                    inst.sync_info = mybir.SyncInfo(
                        on_wait=list(waits[k:]), on_update=list(si.on_update))
                new_insts.append(inst)
            bb.instructions = new_insts


def _build_program():
    nc = bass.Bass("TRN2", target_bir_lowering=False, debug=False)

    xim_d = nc.dram_tensor("xim", [97, T, RR, WW], F16, kind="ExternalInput").ap()
    w1_d = nc.dram_tensor("w1", [3, 97, 128], F16, kind="ExternalInput").ap()
    u1_d = nc.dram_tensor("u1", [3, 96, 128], F16, kind="ExternalInput").ap()
    w2_d = nc.dram_tensor("w2", [3, 99, 128], F16, kind="ExternalInput").ap()
    u2_d = nc.dram_tensor("u2", [3, 96, 128], F16, kind="ExternalInput").ap()
    ysi_d = nc.dram_tensor("ysinit", [99, RR, WW], F16, kind="ExternalInput").ap()
    out_d = nc.dram_tensor("out", [T, 128, 640], F16, kind="ExternalOutput").ap()

    with tile.TileContext(nc) as tc:
        with tc.tile_pool(name="const", bufs=1) as constp, \
             tc.tile_pool(name="ysp", bufs=1) as ysp, \
             tc.tile_pool(name="ximp", bufs=3) as ximp, \
             tc.tile_pool(name="zsp", bufs=3) as zsp, \
             tc.tile_pool(name="hp", bufs=3) as hp, \
             tc.tile_pool(name="htp", bufs=4) as htp, \
             tc.tile_pool(name="tmp", bufs=4) as tmpp, \
             tc.tile_pool(name="outp", bufs=3) as outp, \
             tc.tile_pool(name="ps", bufs=8, space="PSUM") as psp:

            # ---- prefetch first inputs before the (many) weight DMAs ----
            xim_t = {}
            for tt in range(2):
                xim_t[tt] = ximp.tile([97, RR, WW], F16, name=f"xim{tt}")
                nc.sync.dma_start(xim_t[tt][:], xim_d[:, tt])

            # ---- constants / weights ----
            wt = {}
            for nm, src, kk in (("w1", w1_d, 97), ("u1", u1_d, 96),
                                ("w2", w2_d, 99), ("u2", u2_d, 96)):  # w1 first
                for dy in range(3):
                    t_ = constp.tile([kk, 128], F16, tag=f"{nm}{dy}")
                    nc.sync.dma_start(t_[:], src[dy])
                    wt[(nm, dy)] = t_
            half_t = constp.tile([128, 1], F32, tag="half")
            nc.vector.memset(half_t[:], 0.5)

            # ---- persistent state ----
            # recurrent im2col ring buffers: 96 data partitions = 3 dx-shifted
            # channel blocks; ys1h additionally has 3 constant mask rows
            # (96:99) that inject BN2's t2 shift + bias2 for layer 2.
            ys = {}
            for i in range(2):
                t_ = ysp.tile([99, RR, WW], F16, tag=f"ys1h{i}")
                # zeros via idle compute engines; only the 3 constant mask
                # rows come over DMA (keeps the early DMA path clear for xim)
                (nc.vector if i == 0 else nc.gpsimd).memset(t_[0:96], 0.0)
                nc.sync.dma_start(t_[96:99], ysi_d[96:99])
                ys[("ys1h", i)] = t_
            for i in range(2):
                t_ = ysp.tile([96, RR, WW], F16, tag=f"ys2h{i}")
                (nc.vector if i == 0 else nc.gpsimd).memset(t_[:], 0.0)
                ys[("ys2h", i)] = t_
            c_st = {}
            for l in (1, 2):
                c_st[l] = ysp.tile([128, 640], F16, tag=f"c{l}", name=f"c{l}")
            h1ring = {}

            def nblocks(l, t):
                # valid region shrinks 1 row per consumed step; compute only
                # the 2-row blocks that are still needed (min 16 = 32 rows)
                need = (40 - t) if l == 1 else (39 - t)
                return (need + 1) // 2

            def conv_step(l, t, ximt):
                """6-tap conv of (layer l, step t) -> psum tiles, pixel-major."""
                if l == 1:
                    xw, xk = "w1", 97
                else:
                    xw, xk = "w2", 99
                    ximt = ys[("ys1h", t % 2)]
                hsrc = ys[(f"ys{l}h", (t + 1) % 2)]
                nb = nblocks(l, t)
                nk = (nb + 3) // 4
                zps = {}
                # x-taps first (no dependence on last step's unfold)
                for k in range(nk):
                    zps[k] = psp.tile([128, 512], F32, name="zps")
                    for b4 in range(min(4, nb - 4 * k)):
                        b = 4 * k + b4
                        o = zps[k][:, 128 * b4:128 * b4 + 128]
                        for dy in range(3):
                            # start marks the WHOLE 2KB psum bank pending-zero,
                            # so it must fire exactly once per bank
                            nc.tensor.matmul(
                                o, ximt[0:xk, 2 * b + dy:2 * b + dy + 2, :],
                                wt[(xw, dy)][:],
                                start=(b4 == 0 and dy == 0),
                                stop=(t == 0 and b4 == min(3, nb - 4 * k - 1)
                                      and dy == 2))
                # recurrent taps
                for k in range(nk):
                    if t > 0:
                        for b4 in range(min(4, nb - 4 * k)):
                            b = 4 * k + b4
                            o = zps[k][:, 128 * b4:128 * b4 + 128]
                            for dy in range(3):
                                nc.tensor.matmul(
                                    o, hsrc[0:96, 2 * b + dy:2 * b + dy + 2, :],
                                    wt[(f"u{l}", dy)][:],
                                    start=False,
                                    stop=(b4 == min(3, nb - 4 * k - 1)
                                          and dy == 2))
                return zps

            def gate_step(l, t, zps):
                """psum tiles -> h [128 q, nb, 128] padded; updates c_st[l]."""
                nb = nblocks(l, t)
                nk = (nb + 3) // 4
                zs = zsp.tile([128, 2560], F16)
                zv = zs[:].rearrange("q (b gc) -> q b gc", gc=128)
                sig = zv[:, 0:nb, 0:96]
                zi, zf = zv[:, 0:nb, 0:32], zv[:, 0:nb, 32:64]
                zo, zc = zv[:, 0:nb, 64:96], zv[:, 0:nb, 96:128]
                # straight-from-PSUM per k-tile: Relu(0.2z+0.5) for i,f,o and
                # tanh for the c-gate; no PSUM->SBUF copy pass at all
                for k in range(nk):
                    kb = min(4, nb - 4 * k)
                    pv = zps[k][:].rearrange("q (b4 gc) -> q b4 gc", gc=128)
                    nc.scalar.activation(zv[:, 4 * k:4 * k + kb, 0:96],
                                         pv[:, 0:kb, 0:96], AF.Relu,
                                         bias=half_t[:], scale=0.2)
                    nc.scalar.activation(zv[:, 4 * k:4 * k + kb, 96:128],
                                         pv[:, 0:kb, 96:128], AF.Tanh)
                cl = c_st[l]
                cvf = cl[:].rearrange("q (b c) -> q b c", c=32)
                t1 = tmpp.tile([128, 640], F16, tag="t1")
                t1vf = t1[:].rearrange("q (b c) -> q b c", c=32)
                # per-k pipeline: min/clamp + gate muls start as soon as each
                # k-tile's activations land
                h = hp.tile([128, NB, 128], F16, tag=f"h{l}")
                th = tmpp.tile([128, 640], F16, tag="th")
                thvf = th[:].rearrange("q (b c) -> q b c", c=32)
                for k in range((nb + 3) // 4):
                    ks = slice(4 * k, min(nb, 4 * k + 4))
                    sg = zv[:, ks, 0:96]
                    nc.vector.tensor_scalar(sg, sg, 1.0, 0.0,
                                            mybir.AluOpType.min,
                                            mybir.AluOpType.max)
                    if t == 0:
                        nc.vector.tensor_mul(cvf[:, ks], zv[:, ks, 0:32],
                                             zv[:, ks, 96:128])  # c = i*g
                    else:
                        nc.vector.tensor_mul(t1vf[:, ks], zv[:, ks, 0:32],
                                             zv[:, ks, 96:128])  # i*g
                        nc.vector.tensor_mul(cvf[:, ks], cvf[:, ks],
                                             zv[:, ks, 32:64])   # f*c
                        nc.vector.tensor_add(cvf[:, ks], cvf[:, ks],
                                             t1vf[:, ks])        # c new
                    # finish this k-tile end-to-end so the transpose isn't
                    # gated behind whole-width tanh/h ops
                    nc.scalar.activation(thvf[:, ks], cvf[:, ks], AF.Tanh)
                    nc.vector.tensor_mul(h[:, ks, 0:32], zv[:, ks, 64:96],
                                         thvf[:, ks])            # o*tanh(c)
                cv = cvf[:, 0:nb]
                return h

            def unfold(h_t, dst, nb):
                """h [128,nb,128] padded -> XBAR transpose to channel-major
                -> 3 shifted im2col blocks of dst."""
                hT = htp.tile([128, NB, 128], F16)
                nc.sync.dma_start_transpose(
                    hT[:, 0:nb, :],
                    h_t[:, 0:nb, :].rearrange("q b c -> q (b c)"))
                nr = 2 * nb
                hw = hT[0:32, 0:nb].rearrange("c b q -> c (b q)").rearrange(
                    "c (r w) -> c r w", w=64)                    # [32, nr, 64]
                nc.sync.dma_start(dst[0:32, 1:nr + 1, 1:64], hw[:, :, 0:63])
                nc.sync.dma_start(dst[32:64, 1:nr + 1, 0:64], hw)
                nc.sync.dma_start(dst[64:96, 1:nr + 1, 0:63], hw[:, :, 1:64])

            def step(l, t):
                if l == 1:
                    # prefetch the NEXT step's input while this one runs
                    if t + 2 < T:
                        xim_t[t + 2] = ximp.tile([97, RR, WW], F16,
                                                 name="ximn")
                        nc.sync.dma_start(xim_t[t + 2][:], xim_d[:, t + 2])
                    ximt = xim_t[t]
                else:
                    ximt = None
                zps = conv_step(l, t, ximt)
                h = gate_step(l, t, zps)
                if l == 1:
                    unfold(h, ys[("ys1h", t % 2)], nblocks(1, t))
                    h1ring[t % 2] = h
                else:
                    if t < T - 1:
                        unfold(h, ys[("ys2h", t % 2)], nblocks(2, t))
                    # only blocks 0:16 (32 rows) are final output
                    res = outp.tile([128, 640], F16)
                    nc.vector.tensor_add(
                        res[:].rearrange("q (b c) -> q b c", c=32)[:, 0:16],
                        h[:, 0:16, 0:32],
                        h1ring[t % 2][:, 0:16, 0:32])
                    nc.sync.dma_start(out_d[t], res[:])

            for s in range(T + 1):
                if s < T:
                    step(1, s)
                if s >= 1:
                    step(2, s - 1)

    _split_excess_waits(nc)
    return nc


def _host_prep(x, bn_gamma, bn_beta, bn_mean, bn_var, kernels, rec_kernels,
               biases):
    """Build the 8 per-core input maps."""
    # gate reorder [i,f,c,o] -> [i,f,o,c]
    perm = np.concatenate([np.arange(0, 64), np.arange(96, 128),
                           np.arange(64, 96)])
    s1 = bn_gamma[0] / np.sqrt(bn_var[0] + BN_EPS)
    t1 = bn_beta[0] - bn_mean[0] * s1
    s2 = bn_gamma[1] / np.sqrt(bn_var[1] + BN_EPS)
    t2 = bn_beta[1] - bn_mean[1] * s2
    y1 = x * s1 + t1                                  # BN1 on host

    def wmat1(wk, bias_vec, flip):
        """[3,3,C,4F] -> per-dy lhsT-for-rhs [97,128] (+bias row on dy=1)."""
        wk = wk[::-1] if flip else wk
        out = np.zeros((3, 97, 128), np.float32)
        for dy in range(3):
            out[dy, :96] = wk[dy].reshape(96, 128)[:, perm]
        out[1, 96] = bias_vec[perm]
        return out

    def wmat2(wk, bias_vec, flip):
        """Layer-2 input conv with BN2 folded in: rows 0:96 scaled by s2
        (per input channel), rows 96:99 = t2 mask-row weights, bias2 on the
        center-tap mask row (dy=1, k=1)."""
        wk = wk[::-1] if flip else wk
        out = np.zeros((3, 99, 128), np.float32)
        for dy in range(3):
            out[dy, :96] = (wk[dy] * s2[None, :, None]).reshape(
                96, 128)[:, perm]
            for k in range(3):
                out[dy, 96 + k] = (wk[dy, k] * t2[:, None]).sum(0)[perm]
        out[1, 97] += bias_vec[perm]
        return out

    def umat(wk, flip):
        wk = wk[::-1] if flip else wk
        out = np.zeros((3, 96, 128), np.float32)
        for dy in range(3):
            out[dy] = wk[dy].reshape(96, 128)[:, perm]
        return out

    ysinit = np.zeros((99, RR, WW), np.float32)
    ysinit[96, 1:41, 1:64] = 1.0
    ysinit[97, 1:41, 0:64] = 1.0
    ysinit[98, 1:41, 0:63] = 1.0

    in_maps = []
    for core in range(NCORES):
        b, half = core // 2, core % 2
        yb = y1[b] if half == 0 else y1[b, :, ::-1]
        yb = np.ascontiguousarray(yb[:, :RD])         # [T, 41, 64, 32]
        xim = np.zeros((97, T, RR, WW), np.float32)
        yt = yb.transpose(3, 0, 1, 2)                 # [32, T, 41, 64]
        xim[0:32, :, 1:RR, 1:64] = yt[:, :, :, 0:63]
        xim[32:64, :, 1:RR, 0:64] = yt
        xim[64:96, :, 1:RR, 0:63] = yt[:, :, :, 1:64]
        xim[96] = 1.0
        flip = half == 1
        in_maps.append({
            "xim": xim.astype(np.float16),
            "w1": wmat1(kernels[0], biases[0], flip).astype(np.float16),
            "u1": umat(rec_kernels[0], flip).astype(np.float16),
            "w2": wmat2(kernels[1], biases[1], flip).astype(np.float16),
            "u2": umat(rec_kernels[1], flip).astype(np.float16),
            "ysinit": ysinit.astype(np.float16),
        })
    return in_maps


def _decode(results):
    """Per-core out [T,128,640] pixel-major -> full [B,T,H,W,C]."""
    out = np.zeros((B, T, H, W, C), np.float32)
    for core in range(NCORES):
        o = np.asarray(results[core]["out"], np.float32)  # [T, 128, 640]
        o = o.reshape(T, 2, 64, NB, 32)     # [t, dr, w, b, c]
        o = o.transpose(0, 3, 1, 2, 4).reshape(T, R, W, C)  # r = 2b + dr
        b, half = core // 2, core % 2
        if half == 0:
            out[b, :, 0:32] = o[:, 0:32]
        else:
            out[b, :, 32:64] = o[:, 0:32][:, ::-1]
    return out


def kernel(**inputs):
    if "nc" not in _PROG:
        _PROG["nc"] = _build_program()
    in_maps = _host_prep(**inputs)
    res = run_bass_kernel_spmd(_PROG["nc"], in_maps, list(range(NCORES)))
    return _decode(res.results)
